# revision 11
# baseline (speedup 1.0000x reference)
"""Trainium2 Bass kernel for nn_MoE_83786222011162 (moe_routing).

Strategy (8 NeuronCores, SPMD, no collectives):
  - Host computes top-1 gating / capacity routing (it is data-dependent and
    defines the sharding itself): expert-parallel dispatch.
  - Core e receives: the [capacity=1024, H] dispatched token buffer for
    expert e (chain A) plus a 1024-token data-parallel shard for the
    residual FFN branch (chain B), both pre-transposed to [H, tokens];
    w1[e]/res_w1 in fp32 (consumed as float32r by the PE — full-rate fp32
    matmul), w2[e]/res_w2 in bf16; and a per-token output scale
    (gate*coef0 for chain A, coef1 for chain B) folded in on device.
  - Device: hT[f,tok] = gelu_tanh(w1^T x) via f32r matmuls (natural
    layouts, zero transposes), then out[tok, m] = hT^T w2 via bf16
    matmuls, scaled per token (per-partition) and DMA'd out token-major.
  - Host scatters expert rows back by slot, adds the residual shard rows.
"""

import sys

if "/opt/trn_rl_repo" not in sys.path:
    sys.path.insert(0, "/opt/trn_rl_repo")

import numpy as np
import ml_dtypes

H, FF, E = 1024, 4096, 8
B, S = 4, 2048
N_TOK = B * S            # 8192
CAP = 1024               # max(4, int(1.0 * 8192 / 8))
NCORES = 8
SHARD = N_TOK // NCORES  # 1024

_CACHE: dict = {}


def build_moe_bass(h=H, ff=FF, tok=2 * SHARD, th=512, mb=512, g=4,
                   po_bufs=None, ht_bufs=3, w1b_bufs=3, w2b_bufs=8,
                   ph_bufs=3, ob_bufs=4, xv_bufs=4, reps=1):
    """Build the per-core Bass program.

    h: model dim; ff: FFN dim; tok: tokens per core (2 chains);
    th: phase-A token block (<=512, fp32 moving-operand limit);
    mb: phase-B output-column block (<=512, one PSUM bank);
    g: how many 128-token PSUM accumulators run concurrently in phase B.
    """
    from concourse import bacc
    import concourse.mybir as mybir
    from concourse.tile import TileContext

    f32 = mybir.dt.float32
    f32r = mybir.dt.float32r
    bf16 = mybir.dt.bfloat16
    P = 128
    NCH = 2                 # chains per core: expert FFN + residual FFN
    NT = tok // NCH         # tokens per chain
    KT = h // P             # contraction tiles for mm1
    FT = ff // P            # FF tiles
    NTH = NT // th          # phase-A token blocks
    NMH = h // mb           # phase-B output-column blocks
    NTK = NT // P           # 128-token tiles per chain
    act = mybir.ActivationFunctionType

    nc = bacc.Bacc()
    xT = nc.declare_dram_parameter("xT", [h, tok], f32r, isOutput=False)
    w1 = nc.declare_dram_parameter("w1", [NCH, h, ff], f32r, isOutput=False)
    w2 = nc.declare_dram_parameter("w2", [NCH, ff, h], bf16, isOutput=False)
    sc = nc.declare_dram_parameter("sc", [P, tok // P], f32, isOutput=False)
    out = nc.declare_dram_parameter("out", [tok, h], f32, isOutput=True)

    if po_bufs is None:
        po_bufs = min(g + 1, 8 - 3)

    with TileContext(nc) as tc:
        with tc.tile_pool(name="consts", bufs=1) as consts, \
             tc.tile_pool(name="xv", bufs=xv_bufs) as xv_pool, \
             tc.tile_pool(name="w1b", bufs=w1b_bufs) as w1b_pool, \
             tc.tile_pool(name="w2b", bufs=w2b_bufs) as w2b_pool, \
             tc.tile_pool(name="ht", bufs=ht_bufs) as ht_pool, \
             tc.tile_pool(name="ob", bufs=ob_bufs) as ob_pool, \
             tc.tile_pool(name="ph", bufs=ph_bufs, space="PSUM") as ph_pool, \
             tc.tile_pool(name="po", bufs=po_bufs, space="PSUM") as po_pool:

            scale_t = consts.tile([P, tok // P], f32)
            nc.sync.dma_start(out=scale_t, in_=sc[:, :])

            for c in [ch for _ in range(reps) for ch in range(NCH)]:
                # xv loaded per token-half so phase A starts sooner; hT kept
                # as one tile per token-half so chain c+1's phase A can reuse
                # half 0's slot as soon as this chain's phase B is done with
                # it (token-group-outer loop below).
                xvs = []
                hts = []
                for t in range(NTH):
                    xv = xv_pool.tile([P, KT, th], f32r, name="xv", tag="xv")
                    nc.sync.dma_start(
                        out=xv,
                        in_=xT[:, c * NT + t * th:c * NT + (t + 1) * th]
                        .rearrange("(kt p) n -> p kt n", p=P),
                    )
                    xvs.append(xv)
                    hts.append(ht_pool.tile([P, FT, th], bf16,
                                            name="ht", tag="ht"))

                # --- phase A: hT[f, tok] = gelu(w1^T @ x), f32r matmuls ---
                for f in range(FT):
                    w1b = w1b_pool.tile([P, KT, P], f32r)
                    nc.sync.dma_start(
                        out=w1b,
                        in_=w1[c][:, f * P:(f + 1) * P].rearrange(
                            "(kt p) m -> p kt m", p=P),
                    )
                    for t in range(NTH):
                        ph = ph_pool.tile([P, th], f32)
                        for kt in range(KT):
                            nc.tensor.matmul(
                                ph,
                                w1b[:, kt, :],
                                xvs[t][:, kt, :],
                                start=(kt == 0),
                                stop=(kt == KT - 1),
                            )
                        nc.scalar.activation(
                            out=hts[t][:, f, :],
                            in_=ph,
                            func=act.Gelu_apprx_tanh,
                        )

                # --- phase B: out[tok, m] = hT^T @ w2, bf16 matmuls ---
                # token groups outer so hT halves release early for the next
                # chain; m-halves inner (w2 re-streamed per token group).
                tpg = th // P  # 128-token tiles per hT half
                for tg in range(0, NTK, g):
                    for mh in range(NMH):
                        tks = list(range(tg, min(tg + g, NTK)))
                        pos = [po_pool.tile([P, mb], f32, name="po", tag="po")
                               for _ in tks]
                        for f in range(FT):
                            w2b = w2b_pool.tile([P, mb], bf16)
                            nc.sync.dma_start(
                                out=w2b,
                                in_=w2[c][f * P:(f + 1) * P,
                                          mh * mb:(mh + 1) * mb],
                            )
                            for i, tk in enumerate(tks):
                                nc.tensor.matmul(
                                    pos[i],
                                    hts[tk // tpg][:, f,
                                                   (tk % tpg) * P:
                                                   (tk % tpg + 1) * P],
                                    w2b,
                                    start=(f == 0),
                                    stop=(f == FT - 1),
                                )
                        for i, tk in enumerate(tks):
                            ob = ob_pool.tile([P, mb], f32)
                            j = c * NTK + tk
                            nc.vector.tensor_scalar_mul(
                                ob, pos[i], scale_t[:, j:j + 1])
                            nc.sync.dma_start(
                                out=out[c * NT + tk * P:c * NT + (tk + 1) * P,
                                        mh * mb:(mh + 1) * mb],
                                in_=ob,
                            )
    nc.finalize()
    return nc


def _gating(tokens, wg, coef_w, coef_b):
    """Gate probabilities / coef blend, matching the jax-CPU reference
    bit-for-bit when jax is importable (the argmax must not flip)."""
    try:
        import jax
        import jax.numpy as jnp
        cpu = jax.devices("cpu")[0]
        with jax.default_device(cpu):
            tj = jax.device_put(tokens, cpu)
            logits = jnp.matmul(tj, jax.device_put(wg, cpu))
            gates = jax.nn.softmax(logits, axis=1)
            idx = jnp.argmax(gates, axis=1)
            coef = jax.nn.softmax(
                jnp.matmul(tj, jax.device_put(coef_w, cpu))
                + jax.device_put(coef_b, cpu), axis=-1)
            return (np.asarray(gates), np.asarray(idx).astype(np.int64),
                    np.asarray(coef))
    except Exception:
        logits = tokens @ wg
        m = logits.max(axis=1, keepdims=True)
        e = np.exp(logits - m)
        gates = e / e.sum(axis=1, keepdims=True)
        idx = np.argmax(gates, axis=1)
        cl = tokens @ coef_w + coef_b
        m2 = cl.max(axis=1, keepdims=True)
        e2 = np.exp(cl - m2)
        coef = e2 / e2.sum(axis=1, keepdims=True)
        return gates.astype(np.float32), idx, coef.astype(np.float32)


def kernel(hidden_states, wg, w1, w2, res_w1, res_w2, coef_w, coef_b):
    hidden_states = np.ascontiguousarray(np.asarray(hidden_states, np.float32))
    wg = np.ascontiguousarray(np.asarray(wg, np.float32))
    w1 = np.ascontiguousarray(np.asarray(w1, np.float32))
    w2 = np.ascontiguousarray(np.asarray(w2, np.float32))
    res_w1 = np.ascontiguousarray(np.asarray(res_w1, np.float32))
    res_w2 = np.ascontiguousarray(np.asarray(res_w2, np.float32))
    coef_w = np.ascontiguousarray(np.asarray(coef_w, np.float32))
    coef_b = np.ascontiguousarray(np.asarray(coef_b, np.float32))

    b, s, d = hidden_states.shape
    tokens = hidden_states.reshape(-1, d)
    n = tokens.shape[0]

    gates, idx, coef = _gating(tokens, wg, coef_w, coef_b)

    # --- routing (integer arithmetic, exact) ---
    onehot = (idx[:, None] == np.arange(E)[None, :])
    counts = onehot.sum(axis=0).astype(np.int32)          # pre-drop counts
    locs = onehot.cumsum(axis=0) - 1                      # [n, E]
    pos = locs[np.arange(n), idx]                         # slot within expert
    kept = pos < CAP

    exp_counts = counts.astype(np.int32)
    me = gates.mean(axis=0, dtype=np.float32)
    ce = (counts.astype(np.float32) / np.float32(n))
    l_aux = np.float32(np.sum(me * ce, dtype=np.float32) * np.float32(E))

    # --- per-core device inputs ---
    bf = ml_dtypes.bfloat16
    from concourse.bass_utils import run_bass_kernel_spmd

    if "nc" not in _CACHE:
        _CACHE["nc"] = build_moe_bass()
    nc = _CACHE["nc"]

    in_maps = []
    tok_lists = []
    slot_lists = []
    for e in range(E):
        sel = np.where((idx == e) & kept)[0]
        slots = pos[sel]
        disp = np.zeros((CAP, d), np.float32)
        disp[slots] = tokens[sel]
        shard = tokens[e * SHARD:(e + 1) * SHARD]
        xTc = np.ascontiguousarray(
            np.concatenate([disp, shard], axis=0).T)     # [H, 2048]
        scale = np.zeros(2 * SHARD, np.float32)
        scale[slots] = gates[sel, e] * coef[sel, 0]
        scale[SHARD:] = coef[e * SHARD:(e + 1) * SHARD, 1]
        sc2d = np.ascontiguousarray(
            scale.reshape(2 * SHARD // 128, 128).T)      # [128, 16]
        w1c = np.ascontiguousarray(np.stack([w1[e], res_w1]))
        w2c = np.ascontiguousarray(
            np.stack([w2[e], res_w2]).astype(bf))
        in_maps.append({"xT": xTc, "w1": w1c, "w2": w2c, "sc": sc2d})
        tok_lists.append(sel)
        slot_lists.append(slots)

    res = run_bass_kernel_spmd(nc, in_maps, list(range(NCORES)),
                               **_CACHE.get("run_kwargs", {}))
    _CACHE["last_results"] = res

    # --- host combine: residual rows first, then scatter expert rows ---
    outf = np.zeros((n, d), np.float32)
    for e in range(E):
        core_out = res.results[e]["out"]                 # [2048, H]
        outf[e * SHARD:(e + 1) * SHARD] = core_out[SHARD:]
    for e in range(E):
        core_out = res.results[e]["out"]
        sel = tok_lists[e]
        if len(sel):
            outf[sel] += core_out[slot_lists[e]]
    out = outf.reshape(b, s, d)
    return out, l_aux, exp_counts


# revision 14
# speedup vs baseline: 1.0465x; 1.0465x over previous
"""Trainium2 Bass kernel for nn_MoE_83786222011162 (moe_routing).

Strategy (8 NeuronCores, SPMD, no collectives):
  - Host computes top-1 gating / capacity routing (it is data-dependent and
    defines the sharding itself): expert-parallel dispatch.
  - Core e receives: the [capacity=1024, H] dispatched token buffer for
    expert e (chain A) plus a 1024-token data-parallel shard for the
    residual FFN branch (chain B), both pre-transposed to [H, tokens];
    w1[e]/res_w1 in fp32 (consumed as float32r by the PE — full-rate fp32
    matmul), w2[e]/res_w2 in bf16; and a per-token output scale
    (gate*coef0 for chain A, coef1 for chain B) folded in on device.
  - Device: hT[f,tok] = gelu_tanh(w1^T x) via f32r matmuls (natural
    layouts, zero transposes), then out[tok, m] = hT^T w2 via bf16
    matmuls, scaled per token (per-partition) and DMA'd out token-major.
  - Host scatters expert rows back by slot, adds the residual shard rows.
"""

import sys

if "/opt/trn_rl_repo" not in sys.path:
    sys.path.insert(0, "/opt/trn_rl_repo")

import numpy as np
import ml_dtypes

H, FF, E = 1024, 4096, 8
B, S = 4, 2048
N_TOK = B * S            # 8192
CAP = 1024               # max(4, int(1.0 * 8192 / 8))
NCORES = 8
SHARD = N_TOK // NCORES  # 1024

_CACHE: dict = {}


def build_moe_bass(h=H, ff=FF, tok=2 * SHARD, th=512, mb=512, g=4,
                   po_bufs=None, ht_bufs=3, w1b_bufs=3, w2b_bufs=8,
                   ph_bufs=2, ob_bufs=4, xv_bufs=4, reps=1):
    """Build the per-core Bass program.

    h: model dim; ff: FFN dim; tok: tokens per core (2 chains);
    th: phase-A token block (<=512, fp32 moving-operand limit);
    mb: phase-B output-column block (<=512, one PSUM bank);
    g: how many 128-token PSUM accumulators run concurrently in phase B.
    """
    from concourse import bacc
    import concourse.mybir as mybir
    from concourse.tile import TileContext

    f32 = mybir.dt.float32
    f32r = mybir.dt.float32r
    bf16 = mybir.dt.bfloat16
    P = 128
    NCH = 2                 # chains per core: expert FFN + residual FFN
    NT = tok // NCH         # tokens per chain
    KT = h // P             # contraction tiles for mm1
    FT = ff // P            # FF tiles
    NTH = NT // th          # phase-A token blocks
    NMH = h // mb           # phase-B output-column blocks
    NTK = NT // P           # 128-token tiles per chain
    act = mybir.ActivationFunctionType

    nc = bacc.Bacc()
    xT = nc.declare_dram_parameter("xT", [h, tok], f32r, isOutput=False)
    w1 = nc.declare_dram_parameter("w1", [NCH, h, ff], f32r, isOutput=False)
    w2 = nc.declare_dram_parameter("w2", [NCH, ff, h], bf16, isOutput=False)
    sc = nc.declare_dram_parameter("sc", [P, tok // P], f32, isOutput=False)
    out = nc.declare_dram_parameter("out", [tok, h], f32, isOutput=True)

    if po_bufs is None:
        po_bufs = min(g + 2, 8 - ph_bufs)

    with TileContext(nc) as tc:
        with tc.tile_pool(name="consts", bufs=1) as consts, \
             tc.tile_pool(name="xv", bufs=xv_bufs) as xv_pool, \
             tc.tile_pool(name="w1b", bufs=w1b_bufs) as w1b_pool, \
             tc.tile_pool(name="w2b", bufs=w2b_bufs) as w2b_pool, \
             tc.tile_pool(name="ht", bufs=ht_bufs) as ht_pool, \
             tc.tile_pool(name="ob", bufs=ob_bufs) as ob_pool, \
             tc.tile_pool(name="ph", bufs=ph_bufs, space="PSUM") as ph_pool, \
             tc.tile_pool(name="po", bufs=po_bufs, space="PSUM") as po_pool:

            scale_t = consts.tile([P, tok // P], f32)
            nc.sync.dma_start(out=scale_t, in_=sc[:, :])

            for c in [ch for _ in range(reps) for ch in range(NCH)]:
                # xv loaded per token-half so phase A starts sooner; hT kept
                # as one tile per token-half so chain c+1's phase A can reuse
                # half 0's slot as soon as this chain's phase B is done with
                # it (token-group-outer loop below).
                xvs = []
                hts = []
                for t in range(NTH):
                    xv = xv_pool.tile([P, KT, th], f32r, name="xv", tag="xv")
                    # per-kt chunks so the first matmul group can start
                    # streaming behind the DMA instead of waiting for the
                    # whole 2 MiB half
                    xsrc = xT[:, c * NT + t * th:c * NT + (t + 1) * th] \
                        .rearrange("(kt p) n -> p kt n", p=P)
                    for kt in range(KT):
                        nc.sync.dma_start(
                            out=xv[:, kt, :], in_=xsrc[:, kt, :])
                    xvs.append(xv)
                    hts.append(ht_pool.tile([P, FT, th], bf16,
                                            name="ht", tag="ht"))

                # --- phase A: hT[f, tok] = gelu(w1^T @ x), f32r matmuls ---
                for f in range(FT):
                    w1b = w1b_pool.tile([P, KT, P], f32r)
                    nc.sync.dma_start(
                        out=w1b,
                        in_=w1[c][:, f * P:(f + 1) * P].rearrange(
                            "(kt p) m -> p kt m", p=P),
                    )
                    for t in range(NTH):
                        ph = ph_pool.tile([P, th], f32)
                        for kt in range(KT):
                            nc.tensor.matmul(
                                ph,
                                w1b[:, kt, :],
                                xvs[t][:, kt, :],
                                start=(kt == 0),
                                stop=(kt == KT - 1),
                            )
                        nc.scalar.activation(
                            out=hts[t][:, f, :],
                            in_=ph,
                            func=act.Gelu_apprx_tanh,
                        )

                # --- phase B: out[tok, m] = hT^T @ w2, bf16 matmuls ---
                # token groups outer so hT halves release early for the next
                # chain; m-halves inner (w2 re-streamed per token group).
                tpg = th // P  # 128-token tiles per hT half
                for tg in range(0, NTK, g):
                    for mh in range(NMH):
                        tks = list(range(tg, min(tg + g, NTK)))
                        pos = [po_pool.tile([P, mb], f32, name="po", tag="po")
                               for _ in tks]
                        for f in range(FT):
                            w2b = w2b_pool.tile([P, mb], bf16)
                            nc.sync.dma_start(
                                out=w2b,
                                in_=w2[c][f * P:(f + 1) * P,
                                          mh * mb:(mh + 1) * mb],
                            )
                            for i, tk in enumerate(tks):
                                nc.tensor.matmul(
                                    pos[i],
                                    hts[tk // tpg][:, f,
                                                   (tk % tpg) * P:
                                                   (tk % tpg + 1) * P],
                                    w2b,
                                    start=(f == 0),
                                    stop=(f == FT - 1),
                                )
                        for i, tk in enumerate(tks):
                            ob = ob_pool.tile([P, mb], f32)
                            j = c * NTK + tk
                            nc.vector.tensor_scalar_mul(
                                ob, pos[i], scale_t[:, j:j + 1])
                            nc.sync.dma_start(
                                out=out[c * NT + tk * P:c * NT + (tk + 1) * P,
                                        mh * mb:(mh + 1) * mb],
                                in_=ob,
                            )
    nc.finalize()
    return nc


def _gating(tokens, wg, coef_w, coef_b):
    """Gate probabilities / coef blend, matching the jax-CPU reference
    bit-for-bit when jax is importable (the argmax must not flip)."""
    try:
        import jax
        import jax.numpy as jnp
        cpu = jax.devices("cpu")[0]
        with jax.default_device(cpu):
            tj = jax.device_put(tokens, cpu)
            logits = jnp.matmul(tj, jax.device_put(wg, cpu))
            gates = jax.nn.softmax(logits, axis=1)
            idx = jnp.argmax(gates, axis=1)
            coef = jax.nn.softmax(
                jnp.matmul(tj, jax.device_put(coef_w, cpu))
                + jax.device_put(coef_b, cpu), axis=-1)
            return (np.asarray(gates), np.asarray(idx).astype(np.int64),
                    np.asarray(coef))
    except Exception:
        logits = tokens @ wg
        m = logits.max(axis=1, keepdims=True)
        e = np.exp(logits - m)
        gates = e / e.sum(axis=1, keepdims=True)
        idx = np.argmax(gates, axis=1)
        cl = tokens @ coef_w + coef_b
        m2 = cl.max(axis=1, keepdims=True)
        e2 = np.exp(cl - m2)
        coef = e2 / e2.sum(axis=1, keepdims=True)
        return gates.astype(np.float32), idx, coef.astype(np.float32)


def kernel(hidden_states, wg, w1, w2, res_w1, res_w2, coef_w, coef_b):
    hidden_states = np.ascontiguousarray(np.asarray(hidden_states, np.float32))
    wg = np.ascontiguousarray(np.asarray(wg, np.float32))
    w1 = np.ascontiguousarray(np.asarray(w1, np.float32))
    w2 = np.ascontiguousarray(np.asarray(w2, np.float32))
    res_w1 = np.ascontiguousarray(np.asarray(res_w1, np.float32))
    res_w2 = np.ascontiguousarray(np.asarray(res_w2, np.float32))
    coef_w = np.ascontiguousarray(np.asarray(coef_w, np.float32))
    coef_b = np.ascontiguousarray(np.asarray(coef_b, np.float32))

    b, s, d = hidden_states.shape
    tokens = hidden_states.reshape(-1, d)
    n = tokens.shape[0]

    gates, idx, coef = _gating(tokens, wg, coef_w, coef_b)

    # --- routing (integer arithmetic, exact) ---
    onehot = (idx[:, None] == np.arange(E)[None, :])
    counts = onehot.sum(axis=0).astype(np.int32)          # pre-drop counts
    locs = onehot.cumsum(axis=0) - 1                      # [n, E]
    pos = locs[np.arange(n), idx]                         # slot within expert
    kept = pos < CAP

    exp_counts = counts.astype(np.int32)
    me = gates.mean(axis=0, dtype=np.float32)
    ce = (counts.astype(np.float32) / np.float32(n))
    l_aux = np.float32(np.sum(me * ce, dtype=np.float32) * np.float32(E))

    # --- per-core device inputs ---
    bf = ml_dtypes.bfloat16
    from concourse.bass_utils import run_bass_kernel_spmd

    if "nc" not in _CACHE:
        _CACHE["nc"] = build_moe_bass()
    nc = _CACHE["nc"]

    in_maps = []
    tok_lists = []
    slot_lists = []
    for e in range(E):
        sel = np.where((idx == e) & kept)[0]
        slots = pos[sel]
        disp = np.zeros((CAP, d), np.float32)
        disp[slots] = tokens[sel]
        shard = tokens[e * SHARD:(e + 1) * SHARD]
        xTc = np.ascontiguousarray(
            np.concatenate([disp, shard], axis=0).T)     # [H, 2048]
        scale = np.zeros(2 * SHARD, np.float32)
        scale[slots] = gates[sel, e] * coef[sel, 0]
        scale[SHARD:] = coef[e * SHARD:(e + 1) * SHARD, 1]
        sc2d = np.ascontiguousarray(
            scale.reshape(2 * SHARD // 128, 128).T)      # [128, 16]
        w1c = np.ascontiguousarray(np.stack([w1[e], res_w1]))
        w2c = np.ascontiguousarray(
            np.stack([w2[e], res_w2]).astype(bf))
        in_maps.append({"xT": xTc, "w1": w1c, "w2": w2c, "sc": sc2d})
        tok_lists.append(sel)
        slot_lists.append(slots)

    res = run_bass_kernel_spmd(nc, in_maps, list(range(NCORES)),
                               **_CACHE.get("run_kwargs", {}))
    _CACHE["last_results"] = res

    # --- host combine: residual rows first, then scatter expert rows ---
    outf = np.zeros((n, d), np.float32)
    for e in range(E):
        core_out = res.results[e]["out"]                 # [2048, H]
        outf[e * SHARD:(e + 1) * SHARD] = core_out[SHARD:]
    for e in range(E):
        core_out = res.results[e]["out"]
        sel = tok_lists[e]
        if len(sel):
            outf[sel] += core_out[slot_lists[e]]
    out = outf.reshape(b, s, d)
    return out, l_aux, exp_counts
